# revision 7
# baseline (speedup 1.0000x reference)
"""GAT (single-head GATConv) forward on 8 Trainium2 NeuronCores — v2.

Gather-free, three-launch design (dst-sharded, edge-parallel):

  Phase 1 (tiny): each core computes, for its 12500-node slice,
    h = x @ W (bf16, with a trailing 1.0 column) and a_src = h@att_src,
    a_dst = h@att_dst (f32).

  Host: sorts messages (edges + self-loops) by destination, buckets them
    into 16-node "quarters" (8 per 128-node window, T columns of 128 edge
    slots each, T chosen from the data), and stages phase-1 outputs
    pre-shuffled into edge-slot order: hg1[p, col] = [h|1](src),
    asrc/adst per slot, dmb = dst - quarter_base in [0,16). Pure index
    shuffling — all value computation happens on the device.

  Phase 2 (main): per column c (128 edge slots on partitions):
    f = exp(leakyrelu(asrc + adst)) (DVE+ACT, batched);
    S[p, o] = f * (dmb == o)  (16-wide one-hot, DVE);
    psW[65, 128nodes] += hg1_col^T @ S_col  (PE, PSUM-accumulated per
    window at static free offsets 16*quarter — col 64 of hg1 is 1.0 so
    row 64 accumulates the softmax denominator).
    Flush per window: recip = 1/denom (DVE), ones-matmul replicates the
    recip row across partitions, out = g * recip + bias. Outputs
    out^T per window, per-node recip, and per-slot f.

  Phase 3 (tiny): alpha = f * recip[dst]  (recip[dst] staged by host
    from phase-2 outputs; pure elementwise multiply + DMA).

  Softmax max-subtraction is unnecessary: logits are bounded (|logit|
  <~ 40) so exp stays well inside f32 range, and softmax is
  shift-invariant — results match the reference to float rounding.
"""

import os
import sys

sys.path.insert(0, "/opt/trn_rl_repo")

import ml_dtypes
import numpy as np

import concourse.bass as bass
import concourse.mybir as mybir
from concourse.bass_utils import run_bass_kernel_spmd
from concourse.masks import make_identity
from concourse.tile import TileContext

F32 = mybir.dt.float32
BF16 = mybir.dt.bfloat16
AF = mybir.ActivationFunctionType
OP = mybir.AluOpType
NPBF16 = ml_dtypes.bfloat16

N_NODES = 100000
IN_CH = 128
HID = 64
NEG_SLOPE = 0.2

C = 8
Q = 16                      # nodes per quarter
NSLICE = 12512              # nodes per core slice (multiple of Q; last core
                            # covers only 100000 - 7*12512 = 12416 real nodes)
QPC = NSLICE // Q           # quarters per core (782)
QPW = 128 // Q              # quarters per 128-node window
WPC = (NSLICE + 127) // 128 # windows per core (98)
NPAD = WPC * 128            # padded nodes per core (12544)
GN = 8                      # windows per processing chunk
CREAL = [min(NSLICE, N_NODES - c * NSLICE) for c in range(C)]

LAST_HW_NS = {"phase1": None, "phase2": None, "phase3": None, "total": None}


def _fix_excess_waits(nc, maxw=1):
    """walrus here rejects >1 sync wait per instruction; split excess waits
    onto NoOps inserted before the offender on the same engine."""
    ctr = 0
    for fn in nc.m.functions:
        for bb in fn.blocks:
            out = []
            for inst in bb.instructions:
                si = inst.sync_info
                if si is not None and si.on_wait and len(si.on_wait) > maxw:
                    waits = list(si.on_wait)
                    excess, keep = waits[:-maxw], waits[-maxw:]
                    for i in range(0, len(excess), maxw):
                        ctr += 1
                        out.append(mybir.InstNoOp(
                            name=f"I-wfix-{ctr}",
                            engine=inst.engine,
                            ins=[], outs=[],
                            sync_info=mybir.SyncInfo(
                                on_wait=excess[i:i + maxw], on_update=[]),
                        ))
                    del si.on_wait[:]
                    si.on_wait.extend(keep)
                out.append(inst)
            bb.instructions[:] = out
    return ctr


def _maybe_install_trace_hook():
    try:
        from antenv.axon_hooks import get_axon_ntff_profile_hook  # noqa: F401
        return True
    except ImportError:
        pass
    try:
        import types
        mod = types.ModuleType("antenv.axon_hooks")
        holder = {"hook": None}
        mod.set_axon_ntff_profile_hook = lambda h: holder.__setitem__("hook", h)
        mod.get_axon_ntff_profile_hook = lambda: holder["hook"]
        sys.modules["antenv.axon_hooks"] = mod
        import antenv
        antenv.axon_hooks = mod
        if "/root/.axon_site" not in sys.path:
            sys.path.insert(0, "/root/.axon_site")
        from trn_agent_boot.trn_boot import _ntff_profile_via_ctypes
        mod.set_axon_ntff_profile_hook(
            _ntff_profile_via_ctypes("/opt/axon/libaxon_pjrt.so"))
        import concourse.bass_utils as bu
        bu.upload_artifacts = lambda tmpdir: str(tmpdir)
        return True
    except Exception:
        return False


# --------------------------------------------------------------------------
def build_phase1():
    nc = bass.Bass()
    x_e = nc.declare_dram_parameter("xT", [IN_CH, NSLICE], F32, isOutput=False)
    w_e = nc.declare_dram_parameter("W", [IN_CH, HID], F32, isOutput=False)
    as_e = nc.declare_dram_parameter("att_src", [HID, 1], F32, isOutput=False)
    ad_e = nc.declare_dram_parameter("att_dst", [HID, 1], F32, isOutput=False)
    h_e = nc.declare_dram_parameter("p1h", [NSLICE, HID + 2], BF16, isOutput=True)
    a_e = nc.declare_dram_parameter("p1a", [NSLICE, 2], F32, isOutput=True)

    B1 = 8  # windows per chunk
    with TileContext(nc) as tc:
        with (
            tc.tile_pool(name="const", bufs=1) as const,
            tc.tile_pool(name="psc", bufs=1, space="PSUM") as psc,
            tc.tile_pool(name="sb", bufs=3) as sb,
            tc.tile_pool(name="mm", bufs=6, space="PSUM") as mm,
        ):
            ident = const.tile([128, 128], F32)
            make_identity(nc, ident[:])
            wt = const.tile([IN_CH, HID], F32)
            nc.sync.dma_start(out=wt[:], in_=w_e[:])
            att_s = const.tile([HID, 1], F32)
            nc.sync.dma_start(out=att_s[:], in_=as_e[:])
            att_d = const.tile([HID, 1], F32)
            nc.sync.dma_start(out=att_d[:], in_=ad_e[:])

            # rhs = [W | W@att_src | W@att_dst]  (128 x 66)
            wtp = psc.tile([HID, 128], F32, tag="t")
            nc.tensor.transpose(out=wtp[:], in_=wt[:], identity=ident[:])
            wts = const.tile([HID, 128], F32)
            nc.vector.tensor_copy(out=wts[:], in_=wtp[:])
            rhs = const.tile([IN_CH, HID + 2], F32)
            nc.vector.tensor_copy(out=rhs[:, 0:HID], in_=wt[:])
            wsp = psc.tile([IN_CH, 1], F32, tag="t")
            nc.tensor.matmul(out=wsp[:], lhsT=wts[:], rhs=att_s[:],
                             start=True, stop=True)
            nc.vector.tensor_copy(out=rhs[:, HID:HID + 1], in_=wsp[:])
            wdp = psc.tile([IN_CH, 1], F32, tag="t")
            nc.tensor.matmul(out=wdp[:], lhsT=wts[:], rhs=att_d[:],
                             start=True, stop=True)
            nc.vector.tensor_copy(out=rhs[:, HID + 1:HID + 2], in_=wdp[:])

            for c0 in range(0, WPC, B1):
                nb = min(B1, WPC - c0)
                n0 = c0 * 128
                nn = min(nb * 128, NSLICE - n0)
                xt = sb.tile([IN_CH, B1 * 128], F32, tag="xt")
                nc.scalar.dma_start(out=xt[:, :nn], in_=x_e[:, n0:n0 + nn])
                hbc = sb.tile([128, B1 * (HID + 2)], BF16, tag="hbc")
                atc = sb.tile([128, B1 * 2], F32, tag="atc")
                for wl in range(nb):
                    rn = min(128, nn - wl * 128)
                    hp = mm.tile([128, HID + 2], F32, tag="h")
                    nc.tensor.matmul(out=hp[:rn, :],
                                     lhsT=xt[:, wl * 128:wl * 128 + rn],
                                     rhs=rhs[:], start=True, stop=True)
                    ofs = wl * (HID + 2)
                    nc.vector.tensor_copy(out=hbc[:rn, ofs:ofs + HID],
                                          in_=hp[:rn, 0:HID])
                    nc.vector.memset(hbc[:, ofs + HID:ofs + HID + 1], 1.0)
                    nc.vector.memset(hbc[:, ofs + HID + 1:ofs + HID + 2], 0.0)
                    nc.vector.tensor_copy(out=atc[:rn, wl * 2:wl * 2 + 2],
                                          in_=hp[:rn, HID:HID + 2])
                if nn == nb * 128:
                    nc.sync.dma_start(
                        out=h_e[n0:n0 + nb * 128]
                            .rearrange("(w p) f -> p w f", p=128),
                        in_=hbc[:, :nb * (HID + 2)]
                            .rearrange("p (w f) -> p w f", f=HID + 2))
                    nc.sync.dma_start(
                        out=a_e[n0:n0 + nb * 128]
                            .rearrange("(w p) f -> p w f", p=128),
                        in_=atc[:, :nb * 2].rearrange("p (w f) -> p w f", f=2))
                else:
                    for wl in range(nb):
                        rn = min(128, nn - wl * 128)
                        m0 = n0 + wl * 128
                        ofs = wl * (HID + 2)
                        nc.sync.dma_start(out=h_e[m0:m0 + rn, :],
                                          in_=hbc[:rn, ofs:ofs + HID + 2])
                        nc.sync.dma_start(out=a_e[m0:m0 + rn, :],
                                          in_=atc[:rn, wl * 2:wl * 2 + 2])

    _fix_excess_waits(nc)
    return nc


# --------------------------------------------------------------------------
def build_phase2(T):
    """T = columns (of 128 edge slots) per 16-node quarter."""
    CPW = QPW * T            # columns per window
    TOTC = WPC * CPW         # columns per core
    MCC = GN * CPW           # columns per chunk

    nc = bass.Bass()
    hg_e = nc.declare_dram_parameter("hg", [128, TOTC * (HID + 1)], BF16,
                                     isOutput=False)
    NCHUNK = (WPC + GN - 1) // GN
    meta_e = nc.declare_dram_parameter("meta", [128, NCHUNK * 3 * GN * CPW],
                                       F32, isOutput=False)
    iota_e = nc.declare_dram_parameter("iotar", [128, Q], F32, isOutput=False)
    bias_e = nc.declare_dram_parameter("biasc", [HID, 1], F32, isOutput=False)
    outw_e = nc.declare_dram_parameter("outw", [WPC, HID, 128], F32,
                                       isOutput=True)
    rec_e = nc.declare_dram_parameter("recn", [WPC, 128], F32, isOutput=True)
    f_e = nc.declare_dram_parameter("fout", [128, TOTC], F32, isOutput=True)

    chunks = []
    w0 = 0
    while w0 < WPC:
        nw = min(GN, WPC - w0)
        chunks.append((w0, nw))
        w0 += nw
    assert len(chunks) == NCHUNK

    with TileContext(nc) as tc:
        with (
            tc.tile_pool(name="const", bufs=1) as const,
            tc.tile_pool(name="io", bufs=2) as io,
            tc.tile_pool(name="wk", bufs=2) as wk,
            tc.tile_pool(name="mmps", bufs=6, space="PSUM") as mmps,
            tc.tile_pool(name="repps", bufs=2, space="PSUM") as repps,
        ):
            iota_c = const.tile([128, Q], F32)
            nc.sync.dma_start(out=iota_c[:], in_=iota_e[:])
            bias_c = const.tile([HID, 1], F32)
            nc.sync.dma_start(out=bias_c[:], in_=bias_e[:])
            ones_r = const.tile([1, HID], F32)
            nc.vector.memset(ones_r[:], 1.0)

            for (w0, nw) in chunks:
                ncc = nw * CPW          # columns this chunk
                c0 = w0 * CPW
                hgt = io.tile([128, MCC * (HID + 1)], BF16, tag="hg")
                nc.scalar.dma_start(
                    out=hgt[:, :ncc * (HID + 1)],
                    in_=hg_e[:, c0 * (HID + 1):(c0 + ncc) * (HID + 1)])
                ci = w0 // GN
                meta = io.tile([128, MCC * 3], F32, tag="meta")
                nc.scalar.dma_start(
                    out=meta[:],
                    in_=meta_e[:, ci * 3 * MCC:(ci + 1) * 3 * MCC])
                dmb = meta[:, 0:ncc]
                asr = meta[:, MCC:MCC + ncc]
                adr = meta[:, 2 * MCC:2 * MCC + ncc]

                # f = exp(leakyrelu(asrc + adst))
                logit = wk.tile([128, MCC], F32, tag="logit")
                nc.vector.tensor_tensor(out=logit[:, :ncc], in0=asr,
                                        in1=adr, op=OP.add)
                lrt = wk.tile([128, MCC], F32, tag="lrt")
                nc.scalar.activation(out=lrt[:, :ncc], in_=logit[:, :ncc],
                                     func=AF.Prelu, alpha=NEG_SLOPE)
                ft = wk.tile([128, MCC], F32, tag="ft")
                nc.scalar.activation(out=ft[:, :ncc], in_=lrt[:, :ncc],
                                     func=AF.Exp)
                nc.sync.dma_start(out=f_e[:, c0:c0 + ncc], in_=ft[:, :ncc])

                # S[p, col, o] = f[p,col] * (dmb[p,col] == o),  o in [0,16)
                s01 = wk.tile([128, MCC * Q], F32, tag="s01")
                s01v = s01[:, :ncc * Q].rearrange("p (c o) -> p c o", o=Q)
                dmbv = dmb.rearrange("p (c o) -> p c o", o=1) \
                    .to_broadcast([128, ncc, Q])
                iov = iota_c[:].rearrange("p (c o) -> p c o", o=Q) \
                    .to_broadcast([128, ncc, Q])
                nc.vector.tensor_tensor(out=s01v, in0=iov, in1=dmbv,
                                        op=OP.is_equal)
                sbf = wk.tile([128, MCC * Q], BF16, tag="sbf")
                sbfv = sbf[:, :ncc * Q].rearrange("p (c o) -> p c o", o=Q)
                fv = ft[:, :ncc].rearrange("p (c o) -> p c o", o=1) \
                    .to_broadcast([128, ncc, Q])
                nc.gpsimd.tensor_tensor(out=sbfv, in0=s01v, in1=fv, op=OP.mult)

                # aggregation + flush in 4-window sub-batches (PSUM holds
                # only ~5 window accumulators + the rep tile)
                SB = 4
                for s0 in range(0, nw, SB):
                    ns = min(SB, nw - s0)
                    psws = []
                    for wl in range(s0, s0 + ns):
                        psw = mmps.tile([HID + 1, 128], F32, tag="mm")
                        psws.append(psw)
                        for q in range(QPW):
                            for t in range(T):
                                col = wl * CPW + q * T + t
                                nc.tensor.matmul(
                                    out=psw[:, Q * q:Q * q + Q],
                                    lhsT=hgt[:, col * (HID + 1):(col + 1) * (HID + 1)],
                                    rhs=sbf[:, col * Q:(col + 1) * Q],
                                    start=(t == 0), stop=(t == T - 1))

                    # recip row = exp(-ln(denom)) on ACT (reads PSUM
                    # directly; DVE reciprocal is ~6.4 ns/elem, 20x slower)
                    lrow = wk.tile([1, SB * 128], F32, tag="lrow")
                    for i in range(ns):
                        nc.scalar.activation(
                            out=lrow[:1, i * 128:i * 128 + 128],
                            in_=psws[i][HID:HID + 1, :], func=AF.Ln)
                    rrow = wk.tile([1, SB * 128], F32, tag="rrow")
                    nc.scalar.activation(out=rrow[:1, :ns * 128],
                                         in_=lrow[:1, :ns * 128],
                                         func=AF.Exp, scale=-1.0)
                    nc.sync.dma_start(
                        out=rec_e[w0 + s0:w0 + s0 + ns]
                            .rearrange("w p -> (w p)")
                            .rearrange("(o n) -> o n", o=1),
                        in_=rrow[:1, :ns * 128])
                    rep = repps.tile([HID, SB * 128], F32, tag="rep")
                    nc.tensor.matmul(out=rep[:, :ns * 128], lhsT=ones_r[:],
                                     rhs=rrow[:1, :ns * 128], start=True,
                                     stop=True)
                    rep_sb = wk.tile([HID, SB * 128], F32, tag="repsb")
                    nc.vector.tensor_copy(out=rep_sb[:, :ns * 128],
                                          in_=rep[:, :ns * 128])

                    # out[feat, node] = g * recip + bias
                    ostc = wk.tile([HID, SB * 128], F32, tag="ostc")
                    for i in range(ns):
                        nc.vector.tensor_tensor(
                            out=ostc[:, i * 128:i * 128 + 128],
                            in0=psws[i][0:HID, :],
                            in1=rep_sb[:, i * 128:i * 128 + 128], op=OP.mult)
                    nc.vector.tensor_scalar_add(ostc[:, :ns * 128],
                                                ostc[:, :ns * 128],
                                                bias_c[:, :1])
                    nc.sync.dma_start(
                        out=outw_e[w0 + s0:w0 + s0 + ns]
                            .rearrange("w f p -> f w p"),
                        in_=ostc[:, :ns * 128]
                            .rearrange("f (w p) -> f w p", p=128))

    _fix_excess_waits(nc)
    return nc


# --------------------------------------------------------------------------
def build_phase3(TOTC):
    nc = bass.Bass()
    f_e = nc.declare_dram_parameter("fin", [128, TOTC], F32, isOutput=False)
    r_e = nc.declare_dram_parameter("rin", [128, TOTC], F32, isOutput=False)
    a_e = nc.declare_dram_parameter("aout", [128, TOTC], F32, isOutput=True)
    CH = 1024
    with TileContext(nc) as tc:
        with tc.tile_pool(name="sb", bufs=3) as sb:
            for c0 in range(0, TOTC, CH):
                cw = min(CH, TOTC - c0)
                ft = sb.tile([128, CH], F32, tag="f")
                nc.sync.dma_start(out=ft[:, :cw], in_=f_e[:, c0:c0 + cw])
                rt = sb.tile([128, CH], F32, tag="r")
                nc.sync.dma_start(out=rt[:, :cw], in_=r_e[:, c0:c0 + cw])
                at = sb.tile([128, CH], F32, tag="a")
                nc.vector.tensor_tensor(out=at[:, :cw], in0=ft[:, :cw],
                                        in1=rt[:, :cw], op=OP.mult)
                nc.sync.dma_start(out=a_e[:, c0:c0 + cw], in_=at[:, :cw])
    _fix_excess_waits(nc)
    return nc


# --------------------------------------------------------------------------
def _host_prep(edge_index):
    """Sort messages by dst, bucket into 16-node quarters, assign edge slots.

    Returns slot assignment (core, col, p) per sorted message plus T.
    Column numbering within a core: col = w*(8T) + q*T + t.
    """
    src = np.asarray(edge_index[0], dtype=np.int64)
    dst = np.asarray(edge_index[1], dtype=np.int64)
    loop = np.arange(N_NODES, dtype=np.int64)
    src_all = np.concatenate([src, loop])
    dst_all = np.concatenate([dst, loop])
    M = src_all.shape[0]
    perm = np.argsort(dst_all, kind="stable")
    s_srt = src_all[perm]
    d_srt = dst_all[perm]

    qid = d_srt // Q                       # global quarter id
    cnt = np.bincount(qid, minlength=(N_NODES + Q - 1) // Q)
    T = int((cnt.max() + 127) // 128)
    qstart = np.zeros(cnt.shape[0] + 1, np.int64)
    np.cumsum(cnt, out=qstart[1:])
    slot = np.arange(M, dtype=np.int64) - qstart[qid]   # slot within quarter
    core = qid // QPC
    ql = qid - core * QPC                  # quarter local [0, 782)
    t = slot // 128
    p = slot - t * 128
    col = ql * T + t                       # column within core [0, TOTC)
    dmb = (d_srt - qid * Q).astype(np.float32)
    return dict(M=M, T=T, perm=perm, s_srt=s_srt, d_srt=d_srt,
                core=core, col=col, p=p, dmb=dmb)


def kernel(x, edge_index, W, att_src, att_dst, bias):
    x = np.ascontiguousarray(np.asarray(x, dtype=np.float32))
    W = np.ascontiguousarray(np.asarray(W, dtype=np.float32))
    att_src = np.asarray(att_src, dtype=np.float32)
    att_dst = np.asarray(att_dst, dtype=np.float32)
    bias = np.asarray(bias, dtype=np.float32)

    trace = bool(int(os.environ.get("GAT_TRACE", "0")))
    if trace:
        trace = _maybe_install_trace_hook()

    hp = _host_prep(edge_index)
    T = hp["T"]
    TOTC = WPC * QPW * T

    # ---- phase 1
    nc1 = build_phase1()
    xpad = np.zeros((C * NSLICE, IN_CH), np.float32)
    xpad[:N_NODES] = x
    in1 = [
        {
            "xT": np.ascontiguousarray(
                xpad[c * NSLICE:(c + 1) * NSLICE].T),
            "W": W,
            "att_src": att_src.reshape(HID, 1).copy(),
            "att_dst": att_dst.reshape(HID, 1).copy(),
        }
        for c in range(C)
    ]
    res1 = run_bass_kernel_spmd(nc1, in1, list(range(C)), trace=trace)
    LAST_HW_NS["phase1"] = res1.exec_time_ns
    h1 = np.concatenate(
        [res1.results[c]["p1h"][:CREAL[c]] for c in range(C)], axis=0)
    a1 = np.concatenate(
        [res1.results[c]["p1a"][:CREAL[c]] for c in range(C)], axis=0)
    a_src_n = a1[:, 0]
    a_dst_n = a1[:, 1]

    # ---- host staging for phase 2 (pure index shuffling)
    core, col, p = hp["core"], hp["col"], hp["p"]
    s_srt, d_srt, dmb_v = hp["s_srt"], hp["d_srt"], hp["dmb"]

    MCC = GN * QPW * T
    nchunk = (TOTC + MCC - 1) // MCC
    hgs = np.zeros((C, 128, TOTC, HID + 1), NPBF16)
    metas = np.zeros((C, 128, TOTC, 3), np.float32)
    metas[:, :, :, 0] = -1.0
    hgs[core, p, col] = h1[s_srt, :HID + 1]
    metas[core, p, col, 0] = dmb_v
    metas[core, p, col, 1] = a_src_n[s_srt]
    metas[core, p, col, 2] = a_dst_n[d_srt]
    # pack meta per chunk: [dmb-block | asr-block | adr-block] of MCC cols each
    metap = np.zeros((C, 128, nchunk, 3, MCC), np.float32)
    metap[:, :, :, 0, :] = -1.0
    mv = metas.reshape(C, 128, TOTC, 3)
    for ch in range(nchunk):
        c0 = ch * MCC
        ncc = min(MCC, TOTC - c0)
        metap[:, :, ch, 0, :ncc] = mv[:, :, c0:c0 + ncc, 0]
        metap[:, :, ch, 1, :ncc] = mv[:, :, c0:c0 + ncc, 1]
        metap[:, :, ch, 2, :ncc] = mv[:, :, c0:c0 + ncc, 2]

    nc2 = build_phase2(T)
    iotar = np.ascontiguousarray(
        np.broadcast_to(np.arange(Q, dtype=np.float32), (128, Q)))
    biasc = np.ascontiguousarray(bias.reshape(HID, 1))
    in2 = []
    for c in range(C):
        in2.append({
            "hg": np.ascontiguousarray(
                hgs[c].reshape(128, TOTC * (HID + 1))),
            "meta": np.ascontiguousarray(
                metap[c].reshape(128, nchunk * 3 * MCC)),
            "iotar": iotar,
            "biasc": biasc,
        })
    res2 = run_bass_kernel_spmd(nc2, in2, list(range(C)), trace=trace)
    LAST_HW_NS["phase2"] = res2.exec_time_ns

    out = np.empty((N_NODES, HID), np.float32)
    recn = np.empty((C, NPAD), np.float32)
    fres = np.empty((C, 128, TOTC), np.float32)
    for c in range(C):
        ow = res2.results[c]["outw"]  # [WPC, HID, 128]
        out[c * NSLICE:c * NSLICE + CREAL[c]] = \
            ow.transpose(0, 2, 1).reshape(NPAD, HID)[:CREAL[c]]
        recn[c] = res2.results[c]["recn"].reshape(NPAD)
        fres[c] = res2.results[c]["fout"]

    # ---- phase 3: alpha = f * recip[dst]
    rins = np.zeros((C, 128, TOTC), np.float32)
    dl = d_srt - core * NSLICE
    rins[core, p, col] = recn[core, dl]
    nc3 = build_phase3(TOTC)
    in3 = [{"fin": np.ascontiguousarray(fres[c]),
            "rin": np.ascontiguousarray(rins[c])} for c in range(C)]
    res3 = run_bass_kernel_spmd(nc3, in3, list(range(C)), trace=trace)
    LAST_HW_NS["phase3"] = res3.exec_time_ns

    if all(LAST_HW_NS[k] is not None for k in ("phase1", "phase2", "phase3")):
        LAST_HW_NS["total"] = (LAST_HW_NS["phase1"] + LAST_HW_NS["phase2"]
                               + LAST_HW_NS["phase3"])

    M = hp["M"]
    av = np.stack([res3.results[c]["aout"] for c in range(C)], axis=0)
    alpha_srt = av[core, p, col]
    alpha = np.empty(M, np.float32)
    alpha[hp["perm"]] = alpha_srt
    return out, alpha[:, None]
